# revision 3
# baseline (speedup 1.0000x reference)
"""Trainium2 Bass kernel for nn_Attention_65128884077225.

Math: the reference module broadcasts scores [B,H,S,1] along the softmax
axis, so every softmax row is constant -> attention weights are exactly
uniform (1/S). Hence z = mean_s(v) broadcast over s, and the whole module
collapses to, per batch b:

    c[b] = (mean_s x[b,s,:]) @ Wv @ Wout + (bv @ Wout + bout)
    out[b,s,:] = c[b]                      (constant across s)

where Wv = qkv_w[:, 2E:3E], bv = qkv_b[2E:3E].

Sharding (TP-style partial sums, per the hint's tensor-parallel option):
8 cores = 4 batches x 2 sequence-halves. Core c reads rows
[h*1024, (h+1)*1024) of x[b], b=c//2, h=c%2, computes its partial
row c_h = (sum_rows x_h / S) @ Wc, and writes

  - o    [1024, 512] fp16: c_h broadcast over its OWN half of the rows
  - crow [1, 512]    fp16: the bare partial row

The host gather forms out[b, half_h] = o(core h) + crow(core 1-h)
broadcast-added in fp32 (the TP unshard; each core's o covers its
output slice exactly once, same partial-sum pattern as summing two
full partials but with half the HBM store traffic).

Device kernel per core:
  - 8 single-tile x loads (fp32 [128, 2 KiB] descriptors) alternate
    between the gpsimd and sync rings so issue starts the moment each
    engine exits the preamble and the two rings stream the HBM port
    together; the fp16 folded weight follows the last x tile on sync,
  - 3 full-width fp32 matmuls on a memset dummy tile start at preamble
    exit and ramp the PE clock (HAM) before the tail matmuls,
  - a serial DVE add-chain (t0+t1, +t2, ..., +t6) tracks arrivals;
    the final add folds t7 and casts to fp16,
  - 4 fp16 matmuls vs a 1/S-vector give column part-sums -> xsumT/S
    [128,4] in PSUM (1/2048 is a power of two: exact, and it keeps the
    unscaled fp16 Wc out of subnormal range),
  - DVE casts PSUM->SBUF fp16,
  - fused crow+broadcast: 4 fp16 matmuls with the xmean chunk
    replicated across 128 lhsT columns (stride-0) accumulate
    xmean @ Wc into every partition of a [128,512] PSUM tile,
  - DVE PSUM->SBUF fp16 cast, then the [1024, 512] store splits
    across the sync and vector rings (stride-0 source); crow rides
    the idle gpsimd ring.

Host only: fold Wc = Wv @ Wout and bc = bv @ Wout + bout (tiny host
GEMM, fp16 cast), shard inputs, broadcast-add the per-core partials.
"""

import sys

import numpy as np

if "/opt/trn_rl_repo" not in sys.path and not any(
    p.endswith("trn_rl_repo") for p in sys.path
):
    sys.path.insert(0, "/opt/trn_rl_repo")

import concourse.bacc as bacc
import concourse.mybir as mybir
import concourse.tile as tile
from concourse.bass_utils import run_bass_kernel_spmd

B, S, E = 4, 2048, 512
N_CORES = 8
P = 128
SH = S // 2            # 1024 input rows per core (half the sequence)
N_HT = SH // P         # 8 row-tiles per core
FP32 = mybir.dt.float32
FP16 = mybir.dt.float16

_CACHE = {}


def build(bias=True):
    """Build + compile the per-core Bass program (same for every core)."""
    key = "nc" if bias else "nc_nb"
    if key in _CACHE:
        return _CACHE[key]
    nc = bacc.Bacc(None, target_bir_lowering=False, enable_partition_id=False)
    x_d = nc.dram_tensor("x", [SH, E], FP32, kind="ExternalInput")
    wc_d = nc.dram_tensor("wc", [E, E], FP16, kind="ExternalInput")
    bc_d = nc.dram_tensor("bc", [E], FP16, kind="ExternalInput") if bias else None
    o_d = nc.dram_tensor("o", [SH, E], FP16, kind="ExternalOutput")
    crow_d = nc.dram_tensor("crow", [1, E], FP16, kind="ExternalOutput")

    with tile.TileContext(nc) as tc:
        with (
            tc.tile_pool(name="xp", bufs=N_HT) as xp,
            tc.tile_pool(name="wp", bufs=1) as wp,
            tc.tile_pool(name="sp", bufs=1) as sp,
            tc.tile_pool(name="ps", bufs=1, space="PSUM") as ps,
        ):
            # constants + PE warm-up fodder, all on the (idle) DVE early
            ones16 = sp.tile([P, 1], FP16, tag="ones16")
            nc.vector.memset(ones16[:], 1.0 / S)
            ones_col = sp.tile([P, 1], FP32, tag="ones_col")
            nc.vector.memset(ones_col[:], 1.0)
            dummy = sp.tile([P, E], FP32, tag="dummy")
            nc.vector.memset(dummy[:], 1.0)

            # x arrives as 8 single row-tiles: partition p holds rows
            # 8p+t (the reduction is permutation-invariant so any
            # row->partition assignment works). Single tiles keep the
            # completion-sem granularity fine enough that the add chain
            # never waits on more than one in-flight tile; alternating
            # the gpsimd and sync rings starts issue at each engine's
            # preamble exit and lets both rings feed the HBM port.
            x_pt = x_d.rearrange("(p t) e -> p t e", t=N_HT)
            tiles = []
            for t in range(N_HT):
                xc = xp.tile([P, 1, E], FP32, tag="xt")
                eng = nc.gpsimd if t % 2 == 0 else nc.sync
                eng.dma_start(xc[:], x_pt[:, t : t + 1, :])
                tiles.append(xc[:, 0, :])

            # fp16 folded weight (and bias) after the x stream on sync so
            # they never rate-share with (and delay) the last x tiles
            wcb = wp.tile([P, 4, E], FP16, tag="wcb")
            nc.sync.dma_start(wcb[:], wc_d.rearrange("(k p) e -> p k e", p=P))
            if bias:
                # bias row replicated across partitions (DRAM-side
                # stride-0) so DVE can add it lane-local later
                bcr = sp.tile([P, E], FP16, tag="bcr")
                nc.sync.dma_start(bcr[:], bc_d[None, :].broadcast_to([P, E]))

            # PE warm-up (HAM): sustained full-width fp32 work starting
            # right at preamble exit ramps the clock to 2.4 GHz so the
            # tail matmuls run at full clock; 3 matmuls (~6 us) end
            # before the colsum needs the PE.
            p_warm = ps.tile([1, E], FP32, tag="warm")
            for _ in range(3):
                nc.tensor.matmul(
                    p_warm[:], ones_col[:], dummy[:], start=True, stop=True
                )

            # serial accumulate t0..t6 on DVE, pipelined with the stream
            # (full-width adds: narrow DVE ops pay a large fixed cost)
            acc = sp.tile([P, E], FP32, tag="acc")
            nc.vector.tensor_add(acc[:], tiles[0], tiles[1])
            for t in range(2, N_HT - 1):
                nc.vector.tensor_add(acc[:], acc[:], tiles[t])
            # the final add casts the finished sum to fp16 (one rounding,
            # ~5e-4 rel): the colsum matmuls then run single-pass
            acc16 = sp.tile([P, E], FP16, tag="acc16")
            nc.vector.tensor_add(acc16[:], acc[:], tiles[N_HT - 1])

            # column sums -> xsum^T/S [128,4] in PSUM
            # (NB: PSUM start=True resets has_written for the whole bank, so
            # only self-contained or strictly consecutive groups are safe)
            p_red = ps.tile([P, 4], FP32, tag="red")
            for c in range(4):
                nc.tensor.matmul(
                    p_red[:, c : c + 1],
                    acc16[:, c * P : (c + 1) * P],
                    ones16[:],
                    start=True,
                    stop=True,
                )

            # PSUM -> SBUF fp16 cast (fast DVE op, scale already applied)
            xsT = sp.tile([P, 4], FP16, tag="xsT")
            nc.vector.tensor_copy(xsT[:], p_red[:])

            # fused crow+broadcast: one 4-matmul accumulation group.
            # lhsT = xmean chunk replicated across 128 columns (stride-0
            # free dim), so out[p,n] = sum_k xmean_k @ Wc_k = crow[n] in
            # every partition.
            p_out = ps.tile([P, E], FP32, tag="pout")
            for k in range(4):
                nc.tensor.matmul(
                    p_out[:],
                    xsT[:, k : k + 1].broadcast_to([P, P]),
                    wcb[:, k, :],
                    start=(k == 0),
                    stop=(k == 3),
                )
            if bias:
                # crow must stay bias-free (the other core adds it to its
                # own half via the host gather exactly once)
                crow_buf = sp.tile([1, E], FP16, tag="crow_buf")
                nc.vector.tensor_copy(crow_buf[:], p_out[0:1, :])
                nc.gpsimd.dma_start(crow_d[:, :], crow_buf[:])
                obuf = sp.tile([P, E], FP16, tag="obuf")
                nc.vector.tensor_add(obuf[:], p_out[:], bcr[:])
            else:
                obuf = sp.tile([P, E], FP16, tag="obuf")
                nc.vector.tensor_copy(obuf[:], p_out[:])
                nc.gpsimd.dma_start(crow_d[:, :], obuf[0:1, :])

            # the [1024,512] fp16 store splits across two rings, each
            # covering 512 output rows via a stride-0 source
            o_t = o_d.rearrange("(p t) e -> p t e", t=N_HT)
            src = obuf[:, None, :].broadcast_to([P, 4, E])
            nc.sync.dma_start(o_t[:, 0:4, :], src)
            nc.scalar.dma_start(o_t[:, 4:8, :], src)

    nc.compile()
    _CACHE[key] = nc
    return nc


def _fold_weights(qkv_w, qkv_b, out_w, out_b):
    wv = np.asarray(qkv_w)[:, 2 * E : 3 * E].astype(np.float64)
    ow = np.asarray(out_w).astype(np.float64)
    wc = (wv @ ow).astype(np.float16)
    bc = (np.asarray(qkv_b)[2 * E : 3 * E].astype(np.float64) @ ow
          + np.asarray(out_b)).astype(np.float16)
    return wc, bc


def _run(inputs, trace=False, **kwargs):
    x = np.ascontiguousarray(np.asarray(inputs["x"], dtype=np.float32))
    wc, bc = _fold_weights(
        inputs["qkv_w"], inputs["qkv_b"], inputs["out_w"], inputs["out_b"]
    )
    # zero bias (the common torch-default case) compiles to a no-bias
    # program: numerically exact, one fewer DVE op + load
    has_bias = bool(np.any(bc != 0))
    nc = build(bias=has_bias)
    in_maps = []
    for c in range(N_CORES):
        m = {
            "x": np.ascontiguousarray(x[c // 2, (c % 2) * SH : (c % 2 + 1) * SH]),
            "wc": wc,
        }
        if has_bias:
            m["bc"] = bc
        in_maps.append(m)
    res = run_bass_kernel_spmd(
        nc, in_maps, core_ids=list(range(N_CORES)), trace=trace, **kwargs
    )
    # TP-style gather: each core holds c_h broadcast over its own half
    # of the rows plus the bare partial row; the complementary core's
    # row is broadcast-added in fp32
    out = np.empty((B, S, E), dtype=np.float32)
    for b in range(B):
        oA = res.results[2 * b]["o"].astype(np.float32)
        oB = res.results[2 * b + 1]["o"].astype(np.float32)
        rA = res.results[2 * b]["crow"][0].astype(np.float32)
        rB = res.results[2 * b + 1]["crow"][0].astype(np.float32)
        out[b, :SH] = oA + rB[None, :]
        out[b, SH:] = oB + rA[None, :]
    return out, res


def kernel(**inputs) -> np.ndarray:
    out, _ = _run(inputs, trace=False)
    return out


# revision 4
# speedup vs baseline: 1.0632x; 1.0632x over previous
"""Trainium2 Bass kernel for nn_Attention_65128884077225.

Math: the reference module broadcasts scores [B,H,S,1] along the softmax
axis, so every softmax row is constant -> attention weights are exactly
uniform (1/S). Hence z = mean_s(v) broadcast over s, and the whole module
collapses to, per batch b:

    c[b] = (mean_s x[b,s,:]) @ Wv @ Wout + (bv @ Wout + bout)
    out[b,s,:] = c[b]                      (constant across s)

where Wv = qkv_w[:, 2E:3E], bv = qkv_b[2E:3E].

Sharding (TP-style partial sums, per the hint's tensor-parallel option):
8 cores = 4 batches x 2 sequence-halves. Core c reads rows
[h*1024, (h+1)*1024) of x[b], b=c//2, h=c%2, computes its partial
row c_h = (sum_rows x_h / S) @ Wc, and writes

  - o    [1024, 512] fp16: c_h broadcast over its OWN half of the rows
  - crow [1, 512]    fp16: the bare partial row

The host gather forms out[b, half_h] = o(core h) + crow(core 1-h)
broadcast-added in fp32 (the TP unshard; each core's o covers its
output slice exactly once, same partial-sum pattern as summing two
full partials but with half the HBM store traffic).

Device kernel per core (x stream on a single HWDGE ring so tiles
arrive in order — the Tile scheduler hoists rate-sharing DMAs to the
front if a ring's ready-heap ever runs dry, so keep the stream
self-contained on one ring with the weight load emitted last):
  - 4 loads of x row-tile pairs + 2 singles (fp32, 4/2 KiB
    descriptors) stream back-to-back on the sync ring; the fp16
    folded weight follows after the last x tile so it never delays it,
  - a tiny primer on the scalar ring warms the SDMA/HBM path before
    the real stream (the first DMAs otherwise ramp slowly),
  - 3 full-width fp32 matmuls on a memset dummy tile start at
    preamble exit and ramp the PE clock (HAM) before the tail
    matmuls need it,
  - a serial DVE add-chain accumulates tiles t0..t7 while the stream
    runs; the final add folds t7 and casts to fp16,
  - 4 fp16 matmuls vs a 1/S-vector give column part-sums -> xsumT/S
    [128,4] in PSUM (1/2048 is a power of two: exact in fp16, and it
    keeps the unscaled fp16 Wc out of subnormal range),
  - DVE casts PSUM->SBUF fp16,
  - fused crow+broadcast: 4 fp16 matmuls with the xmean chunk
    replicated across 128 lhsT columns (stride-0) accumulate
    xmean @ Wc into every partition of a [128,512] PSUM tile,
  - DVE PSUM->SBUF fp16 cast, then the [1024,512] store splits
    across the sync and scalar rings (stride-0 source); the tiny crow
    store leads on the scalar ring and doubles as its queue warmer.

Host only: fold Wc = Wv @ Wout and bc = bv @ Wout + bout (tiny host
GEMM, fp16 cast), shard inputs, broadcast-add the per-core partials.
"""

import sys

import numpy as np

if "/opt/trn_rl_repo" not in sys.path and not any(
    p.endswith("trn_rl_repo") for p in sys.path
):
    sys.path.insert(0, "/opt/trn_rl_repo")

import concourse.bacc as bacc
import concourse.mybir as mybir
import concourse.tile as tile
from concourse.bass_utils import run_bass_kernel_spmd

B, S, E = 4, 2048, 512
N_CORES = 8
P = 128
SH = S // 2            # 1024 input rows per core (half the sequence)
N_HT = SH // P         # 8 row-tiles per core
FP32 = mybir.dt.float32
FP16 = mybir.dt.float16

_CACHE = {}


def build(bias=True):
    """Build + compile the per-core Bass program (same for every core)."""
    key = "nc" if bias else "nc_nb"
    if key in _CACHE:
        return _CACHE[key]
    nc = bacc.Bacc(None, target_bir_lowering=False, enable_partition_id=False)
    x_d = nc.dram_tensor("x", [SH, E], FP32, kind="ExternalInput")
    wc_d = nc.dram_tensor("wc", [E, E], FP16, kind="ExternalInput")
    bc_d = nc.dram_tensor("bc", [E], FP16, kind="ExternalInput") if bias else None
    o_d = nc.dram_tensor("o", [SH, E], FP16, kind="ExternalOutput")
    crow_d = nc.dram_tensor("crow", [1, E], FP16, kind="ExternalOutput")

    with tile.TileContext(nc) as tc:
        with (
            tc.tile_pool(name="xp", bufs=9) as xp,
            tc.tile_pool(name="wp", bufs=1) as wp,
            tc.tile_pool(name="sp", bufs=1) as sp,
            tc.tile_pool(name="ps", bufs=1, space="PSUM") as ps,
        ):
            # constants + PE warm-up fodder, all on the (idle) DVE early
            ones16 = sp.tile([P, 1], FP16, tag="ones16")
            nc.vector.memset(ones16[:], 1.0 / S)
            ones_col = sp.tile([P, 1], FP32, tag="ones_col")
            nc.vector.memset(ones_col[:], 1.0)
            dummy = sp.tile([P, E], FP32, tag="dummy")
            nc.vector.memset(dummy[:], 1.0)

            # tiny primer on the idle scalar ring warms the SDMA/HBM path
            # before the real stream
            primer = sp.tile([4, E], FP16, tag="primer")
            nc.scalar.dma_start(primer[:], wc_d[0:4, :])

            # x arrives as row tiles: partition p holds rows 8p+t (the
            # reduction is permutation-invariant so any row->partition
            # assignment works; pairs give 4 KiB contiguous descriptors).
            # t6/t7 load as singles: their completion sems fire before the
            # (add-bound) chain reaches them, so the last adds never stall
            # on the DMA-completion straggler.
            x_pt = x_d.rearrange("(p t) e -> p t e", t=N_HT)
            groups = [(0, 2), (2, 4), (4, 6), (6, 7), (7, 8)]
            tiles = []
            for lo, hi in groups:
                xc = xp.tile([P, hi - lo, E], FP32, tag="xc")
                nc.sync.dma_start(xc[:], x_pt[:, lo:hi, :])
                for i in range(hi - lo):
                    tiles.append(xc[:, i, :])

            # fp16 folded weight (and bias) after the x stream on sync so
            # they never rate-share with (and delay) the last x tiles
            wcb = wp.tile([P, 4, E], FP16, tag="wcb")
            nc.sync.dma_start(wcb[:], wc_d.rearrange("(k p) e -> p k e", p=P))
            if bias:
                # bias row replicated across partitions (DRAM-side
                # stride-0) so DVE can add it lane-local later
                bcr = sp.tile([P, E], FP16, tag="bcr")
                nc.sync.dma_start(bcr[:], bc_d[None, :].broadcast_to([P, E]))

            # PE warm-up (HAM): sustained full-width fp32 work starting
            # right at preamble exit ramps the clock to 2.4 GHz so the
            # tail matmuls run at full clock; 3 matmuls (~6 us) end
            # before the colsum needs the PE.
            p_warm = ps.tile([1, E], FP32, tag="warm")
            for _ in range(3):
                nc.tensor.matmul(
                    p_warm[:], ones_col[:], dummy[:], start=True, stop=True
                )

            # serial accumulate t0..t6 on DVE, pipelined with the stream
            # (full-width adds: narrow DVE ops pay a large fixed cost)
            acc = sp.tile([P, E], FP32, tag="acc")
            nc.vector.tensor_add(acc[:], tiles[0], tiles[1])
            for t in range(2, N_HT - 1):
                nc.vector.tensor_add(acc[:], acc[:], tiles[t])
            # the final add casts the finished sum to fp16 (one rounding,
            # ~5e-4 rel): the colsum matmuls then run single-pass
            acc16 = sp.tile([P, E], FP16, tag="acc16")
            nc.vector.tensor_add(acc16[:], acc[:], tiles[N_HT - 1])

            # column sums -> xsum^T/S [128,4] in PSUM
            # (NB: PSUM start=True resets has_written for the whole bank, so
            # only self-contained or strictly consecutive groups are safe)
            p_red = ps.tile([P, 4], FP32, tag="red")
            for c in range(4):
                nc.tensor.matmul(
                    p_red[:, c : c + 1],
                    acc16[:, c * P : (c + 1) * P],
                    ones16[:],
                    start=True,
                    stop=True,
                )

            # PSUM -> SBUF fp16 cast (fast DVE op, scale already applied)
            xsT = sp.tile([P, 4], FP16, tag="xsT")
            nc.vector.tensor_copy(xsT[:], p_red[:])

            # fused crow+broadcast: one 4-matmul accumulation group.
            # lhsT = xmean chunk replicated across 128 columns (stride-0
            # free dim), so out[p,n] = sum_k xmean_k @ Wc_k = crow[n] in
            # every partition.
            p_out = ps.tile([P, E], FP32, tag="pout")
            for k in range(4):
                nc.tensor.matmul(
                    p_out[:],
                    xsT[:, k : k + 1].broadcast_to([P, P]),
                    wcb[:, k, :],
                    start=(k == 0),
                    stop=(k == 3),
                )
            if bias:
                # crow must stay bias-free (the other core adds it to its
                # own half via the host gather exactly once)
                crow_buf = sp.tile([1, E], FP16, tag="crow_buf")
                nc.vector.tensor_copy(crow_buf[:], p_out[0:1, :])
                nc.scalar.dma_start(crow_d[:, :], crow_buf[:])
                obuf = sp.tile([P, E], FP16, tag="obuf")
                nc.vector.tensor_add(obuf[:], p_out[:], bcr[:])
            else:
                obuf = sp.tile([P, E], FP16, tag="obuf")
                nc.vector.tensor_copy(obuf[:], p_out[:])
                nc.scalar.dma_start(crow_d[:, :], obuf[0:1, :])

            # the [1024,512] fp16 store splits across the sync and scalar
            # rings, each covering 512 output rows via a stride-0 source
            o_t = o_d.rearrange("(p t) e -> p t e", t=N_HT)
            src = obuf[:, None, :].broadcast_to([P, 4, E])
            nc.sync.dma_start(o_t[:, 0:4, :], src)
            nc.scalar.dma_start(o_t[:, 4:8, :], src)

    nc.compile()
    _CACHE[key] = nc
    return nc


def _fold_weights(qkv_w, qkv_b, out_w, out_b):
    wv = np.asarray(qkv_w)[:, 2 * E : 3 * E].astype(np.float64)
    ow = np.asarray(out_w).astype(np.float64)
    wc = (wv @ ow).astype(np.float16)
    bc = (np.asarray(qkv_b)[2 * E : 3 * E].astype(np.float64) @ ow
          + np.asarray(out_b)).astype(np.float16)
    return wc, bc


def _run(inputs, trace=False, **kwargs):
    x = np.ascontiguousarray(np.asarray(inputs["x"], dtype=np.float32))
    wc, bc = _fold_weights(
        inputs["qkv_w"], inputs["qkv_b"], inputs["out_w"], inputs["out_b"]
    )
    # zero bias (the common torch-default case) compiles to a no-bias
    # program: numerically exact, one fewer DVE op + load
    has_bias = bool(np.any(bc != 0))
    nc = build(bias=has_bias)
    in_maps = []
    for c in range(N_CORES):
        m = {
            "x": np.ascontiguousarray(x[c // 2, (c % 2) * SH : (c % 2 + 1) * SH]),
            "wc": wc,
        }
        if has_bias:
            m["bc"] = bc
        in_maps.append(m)
    res = run_bass_kernel_spmd(
        nc, in_maps, core_ids=list(range(N_CORES)), trace=trace, **kwargs
    )
    # TP-style gather: each core holds c_h broadcast over its own half
    # of the rows plus the bare partial row; the complementary core's
    # row is broadcast-added in fp32
    out = np.empty((B, S, E), dtype=np.float32)
    for b in range(B):
        oA = res.results[2 * b]["o"].astype(np.float32)
        oB = res.results[2 * b + 1]["o"].astype(np.float32)
        rA = res.results[2 * b]["crow"][0].astype(np.float32)
        rB = res.results[2 * b + 1]["crow"][0].astype(np.float32)
        out[b, :SH] = oA + rB[None, :]
        out[b, SH:] = oB + rA[None, :]
    return out, res


def kernel(**inputs) -> np.ndarray:
    out, _ = _run(inputs, trace=False)
    return out
